# revision 31
# baseline (speedup 1.0000x reference)
"""Trainium2 Bass kernel for a pre-LN transformer block (B=4, S=2048, H=12, D=64).

Sharding: 8 cores; core c -> batch b = c//2, parity p = c%2.
Each core handles the 1024 query rows of its batch whose 128-token block index
has parity p (stride-2 interleave balances causal load; SPMD shared program).

v2 layout trick: the host PERMUTES each core's token axis so that the core's
own query tokens are columns 0:1024 and the other parity's tokens are columns
1024:2048.  Q projection / residual / output then just use the first half of
the feature-major activations; causal structure is carried entirely by the
per-core host-computed multiplicative masks (triangular for key tiles 0..7,
all-0 / all-1 for key tiles 8..15 depending on parity).

Other v2 changes vs baseline:
- All weights host-packed into per-partition-contiguous layouts, streamed
  mo-outer so each weight is DMA'd once at high efficiency (W1/W2 prefetched
  into SBUF on the gpsimd queue during attention / Wo).
- LayerNorm stays on-chip: ones-matmul sums -> 1-lane math on [1,512] ->
  outer-product matmul broadcast into PSUM -> DVE apply.  No DRAM bounces.
- Softmax: exp on Scalar with head-pair merged activations when the span fits
  512 cols; denominators from the ones-row in V, broadcast by a row-64
  expand-matmul, normalized by DVE reciprocal+multiply.  No DRAM bounces.
- Residual r kept in bf16; biases folded on host (all zero for this model).
"""

import numpy as np

N_CORES = 8
B, S, H, D = 4, 2048, 12, 64
HID = 768
QL = 1024
KT = HID // 128     # 6 feature blocks
TT = S // 128       # 16 key tiles
MH = 4 * HID // 128  # 24 hidden blocks
EPS = 1e-5

_CACHE = {}


def _build_program(biases_zero, debug=False):
    from contextlib import ExitStack
    import concourse.bass as bass
    import concourse.tile as tile
    from concourse import bacc, mybir

    F32 = mybir.dt.float32
    BF16 = mybir.dt.bfloat16
    Alu = mybir.AluOpType
    Act = mybir.ActivationFunctionType

    nc = bacc.Bacc("TRN2", target_bir_lowering=False, debug=False,
                   enable_asserts=False, num_devices=N_CORES)

    def din(name, shape, dt):
        return nc.dram_tensor(name, shape, dt, kind="ExternalInput").ap()

    # activations / consts (per-core)
    xbT = din("xbT", [128, KT, S], BF16)       # packed, token-permuted
    masks = din("masks", [128, 2, 128], BF16)  # [tri | flat(parity)]
    # weights (shared across cores), host-packed per-partition-contiguous
    Wq = din("Wq", [128, KT, HID], BF16)       # [p][kt][m]; ln1_w and 1/8 folded
    Wk = din("Wk", [128, KT, HID], BF16)       # ln1_w folded
    Wv = din("Wv", [128, KT, HID], BF16)       # ln1_w folded
    Wo = din("Wo", [128, KT, HID], BF16)
    W1 = din("W1", [MH, 128, KT, 128], BF16)   # [mo][p][kt][cols]; ln2_w folded
    W2 = din("W2", [MH, 128, HID], BF16)       # [k2][p][m]
    bqs = din("bqs", [HID], F32)
    bk = din("bk", [HID], F32)
    bv = din("bv", [HID], F32)
    bo = din("bo", [HID], F32)
    b1 = din("b1", [4 * HID], F32)
    b2 = din("b2", [HID], F32)

    y = nc.dram_tensor("y", [HID, QL], F32, kind="ExternalOutput").ap()

    def bcast(src_elem_ap, parts, n):
        return bass.AP(tensor=src_elem_ap.tensor, offset=src_elem_ap.offset,
                       ap=[[0, parts], [1, n]])

    with tile.TileContext(nc) as tc, ExitStack() as ctx:
        sb = ctx.enter_context(tc.tile_pool(name="sb", bufs=1))
        ps = ctx.enter_context(tc.tile_pool(name="ps", bufs=1, space="PSUM"))

        def pst_O(name, shape=(65, 1024)):
            return ps.tile(list(shape), F32, tag="sO", bufs=2, name=name,
                           padded_shape=[128, 1024])

        def pst_sc(name, shape=(128, 512)):
            return ps.tile(list(shape), F32, tag="sc", bufs=2, name=name,
                           padded_shape=[128, 512])

        def pst_tile(name, shape=(128, 512)):
            return ps.tile(list(shape), F32, tag="pj", bufs=2, name=name,
                           padded_shape=[128, 512])

        # ---------- constants ----------
        ones_bf = sb.tile([128, 1], BF16, tag="ones")
        nc.vector.memset(ones_bf, 1.0)
        ones_f = sb.tile([128, 1], F32, tag="onesf")
        nc.vector.memset(ones_f, 1.0)
        ones_row = sb.tile([1, 128], BF16, tag="onesr")
        nc.vector.memset(ones_row, 1.0)
        par = sb.tile([128, 80], F32, tag="par")

        def load_cols(dst0, src, n):
            nc.gpsimd.dma_start(
                out=par[:, dst0:dst0 + n],
                in_=bass.AP(tensor=src.tensor, offset=src.offset,
                            ap=[[1, 128], [128, n]]))

        load_cols(24, bqs, KT)
        load_cols(30, bk, KT)
        load_cols(36, bo, KT)
        load_cols(42, b2, KT)
        load_cols(48, b1, MH)
        nc.vector.memset(par[:, 72:73], EPS)
        eps_t = par[:, 72:73]
        if not biases_zero:
            bv_b = sb.tile([128, HID], F32, tag="bv_b")
            nc.gpsimd.dma_start(out=bv_b, in_=bcast(bv[0], 128, HID))
        E_sb = sb.tile([65, 128], BF16, tag="emat")
        nc.vector.memset(E_sb[64:65, :], 1.0)
        masks_sb = sb.tile([128, 2, 128], BF16, tag="masks")
        nc.gpsimd.dma_start(out=masks_sb, in_=masks)

        # PE warmup: lift the HAM clock gate while input DMAs land
        warm = sb.tile([128, 512], BF16, tag="sq", bufs=2, name="warm")
        nc.vector.memset(warm, 0.0)
        wps = pst_tile("warmps", (1, 512))
        for i in range(20):
            nc.tensor.matmul(wps, ones_bf, warm, start=True, stop=True)

        # ---------- input load ----------
        xbT_sb = sb.tile([128, KT, S], BF16, tag="xbT")
        for c in range(S // 512):
            nc.sync.dma_start(out=xbT_sb[:, :, 512 * c:512 * c + 512],
                              in_=xbT[:, :, 512 * c:512 * c + 512])

        # ---------- LN helper: broadcast stats first, then 128-lane math ------
        def emit_ln(N, x_bf, out_bf, pfx, f32_sq=False):
            for c in range(N // 512):
                off = 512 * c
                cs = slice(off, off + 512)
                s_ps = pst_tile(f"{pfx}s{c}", (1, 512))
                q_ps = pst_tile(f"{pfx}q{c}", (1, 512))
                for kt in range(KT):
                    nc.tensor.matmul(s_ps, ones_bf, x_bf[:, kt, cs],
                                     start=(kt == 0), stop=(kt == KT - 1))
                for kt in range(KT):
                    sqd = F32 if f32_sq else BF16
                    sqc = sb.tile([128, 512], sqd, tag="sq", bufs=2,
                                  name=f"{pfx}sq{c}_{kt}")
                    nc.vector.tensor_mul(sqc, x_bf[:, kt, cs], x_bf[:, kt, cs])
                    nc.tensor.matmul(q_ps, ones_f if f32_sq else ones_bf, sqc,
                                     start=(kt == 0), stop=(kt == KT - 1))
                s_b = sb.tile([1, 512], BF16, tag="st1b", bufs=2, name=f"{pfx}sb{c}")
                q_b = sb.tile([1, 512], BF16, tag="st1b", bufs=2, name=f"{pfx}qb{c}")
                nc.scalar.copy(s_b, s_ps)
                nc.scalar.copy(q_b, q_ps)
                S_ps = pst_tile(f"{pfx}S{c}", (128, 512))
                Q_ps = pst_tile(f"{pfx}Q{c}", (128, 512))
                nc.tensor.matmul(S_ps, ones_row, s_b, start=True, stop=True)
                nc.tensor.matmul(Q_ps, ones_row, q_b, start=True, stop=True)
                # full-width math: var = q/H - mu^2; rstd = 1/sqrt(var+eps)
                mu = sb.tile([128, 512], F32, tag="lnm", bufs=4, name=f"{pfx}mu{c}")
                var = sb.tile([128, 512], F32, tag="lnm", bufs=4, name=f"{pfx}v{c}")
                nc.scalar.mul(mu, S_ps, 1.0 / HID)
                nc.scalar.mul(var, Q_ps, 1.0 / HID)
                nmu2 = sb.tile([128, 512], F32, tag="lnm", bufs=4,
                               name=f"{pfx}n2{c}")
                nc.vector.scalar_tensor_tensor(nmu2, mu, -1.0, mu, Alu.mult,
                                               Alu.mult)  # -mu^2
                nc.vector.tensor_add(var, var, nmu2)      # q/H - mu^2
                sd = sb.tile([128, 512], F32, tag="lnm", bufs=4, name=f"{pfx}sd{c}")
                nc.scalar.activation(sd, var, Act.Sqrt, bias=eps_t, scale=1.0)
                a_t = sb.tile([128, 512], F32, tag="lnm", bufs=4, name=f"{pfx}a{c}")
                nc.vector.reciprocal_approx_fast(out=a_t, in_=sd)
                c_t = sb.tile([128, 512], F32, tag="lnm", bufs=4, name=f"{pfx}c{c}")
                nc.vector.scalar_tensor_tensor(c_t, mu, -1.0, a_t, Alu.mult,
                                               Alu.mult)  # -mu * rstd
                for kt in range(KT):
                    t0 = sb.tile([128, 512], BF16, tag="t0", bufs=2,
                                 name=f"{pfx}t0{c}_{kt}")
                    nc.vector.tensor_mul(t0, x_bf[:, kt, cs], a_t)
                    nc.vector.tensor_add(out_bf[:, kt, cs], t0, c_t)

        ln_bf = sb.tile([128, KT, S], BF16, tag="ln")
        emit_ln(S, xbT_sb, ln_bf, "l1")

        # evacuation helper: psum -> sbuf (+ optional bias col)
        def evac(dst, src, bias_col=None):
            if biases_zero or bias_col is None:
                nc.vector.tensor_copy(dst, src)
            else:
                nc.vector.tensor_scalar(dst, src, 1.0, bias_col,
                                        Alu.mult, Alu.add)

        # ---------- QKV projections (mo-outer, weights loaded once) ----------
        K_sb = sb.tile([128, KT, S], BF16, tag="K")
        Q_sb = sb.tile([128, KT, QL], BF16, tag="Q")
        V_sb = sb.tile([128, H, TT, 65], BF16, tag="V")
        for h in range(H):
            nc.vector.memset(V_sb[:, h, :, 64:65], 1.0)

        def emit_kq(mo):
            wkt = sb.tile([128, KT, 128], BF16, tag="wk6", bufs=2,
                          name=f"wk{mo}")
            nc.sync.dma_start(out=wkt, in_=Wk[:, :, 128 * mo:128 * mo + 128])
            for n in range(S // 512):
                cs = slice(512 * n, 512 * n + 512)
                pst = pst_tile(f"kps{mo}_{n}", (128, 512))
                for kt in range(KT):
                    nc.tensor.matmul(pst, wkt[:, kt, :], ln_bf[:, kt, cs],
                                     start=(kt == 0), stop=(kt == KT - 1))
                evac(K_sb[:, mo, cs], pst, par[:, 30 + mo:31 + mo])
            wqt = sb.tile([128, KT, 128], BF16, tag="wk6", bufs=2,
                          name=f"wq{mo}")
            nc.sync.dma_start(out=wqt, in_=Wq[:, :, 128 * mo:128 * mo + 128])
            for n in range(QL // 512):
                cs = slice(512 * n, 512 * n + 512)
                pst = pst_tile(f"qps{mo}_{n}", (128, 512))
                for kt in range(KT):
                    nc.tensor.matmul(pst, wqt[:, kt, :], ln_bf[:, kt, cs],
                                     start=(kt == 0), stop=(kt == KT - 1))
                evac(Q_sb[:, mo, cs], pst, par[:, 24 + mo:25 + mo])

        def emit_v(fc):
            wvt = sb.tile([128, KT, 384], BF16, tag="wv", bufs=1, name=f"wv{fc}")
            nc.sync.dma_start(out=wvt,
                              in_=Wv[:, :, 384 * fc:384 * fc + 384])
            for tt in range(TT):
                pst = pst_tile(f"vps{tt}_{fc}", (128, 384))
                for kt in range(KT):
                    nc.tensor.matmul(pst, ln_bf[:, kt, 128 * tt:128 * tt + 128],
                                     wvt[:, kt, :],
                                     start=(kt == 0), stop=(kt == KT - 1))
                vdst = V_sb[:, 6 * fc:6 * fc + 6, tt, 0:64]
                if biases_zero:
                    nc.vector.tensor_copy(vdst,
                                          pst.rearrange("p (h d) -> p h d", d=64))
                else:
                    nc.vector.tensor_tensor(
                        vdst, pst.rearrange("p (h d) -> p h d", d=64),
                        bv_b[:, 384 * fc:384 * fc + 384].rearrange(
                            "p (h d) -> p h d", d=64),
                        Alu.add)

        W1_sb = sb.tile([128, 12, KT, 128], BF16, tag="W1")

        # ---------- interleaved QKV + attention ----------
        attn_bf = sb.tile([128, KT, QL], BF16, tag="attn")
        for kt in range(KT):
            if kt == 0:
                emit_kq(0)
                emit_v(0)
            if kt < KT - 1:
                emit_kq(kt + 1)   # next pair's K/Q fills exp-wait PE slots
            if kt == 2:
                emit_v(1)
            if kt == 1:
                for mo in range(12):
                    nc.gpsimd.dma_start(out=W1_sb[:, mo, :, :], in_=W1[mo])
            h0, h1 = 2 * kt, 2 * kt + 1
            O = {h0: pst_O(f"o{h0}"), h1: pst_O(f"o{h1}")}

            def av(h, tile_t, e, cs, ce):
                nc.tensor.matmul(O[h][:, cs:ce], V_sb[:, h, tile_t, :],
                                 e, start=(tile_t == 0),
                                 stop=(tile_t == TT - 1))

            prev = []
            for t in range(TT):
                q0 = 128 * (t % 8)
                mk = masks_sb[:, 0 if t < 8 else 1, :]
                cur = []
                for h, pr in ((h0, slice(0, 64)), (h1, slice(64, 128))):
                    for (cs, ce) in ([(q0, 512), (512, QL)] if q0 < 512
                                     else [(q0, QL)]):
                        S_p = pst_sc(f"sc{h}_{t}_{cs}")
                        nc.tensor.matmul(
                            S_p[:, 0:ce - cs],
                            K_sb[pr, kt, 128 * t:128 * t + 128],
                            Q_sb[pr, kt, cs:ce], start=True, stop=True)
                        eS = sb.tile([128, ce - cs], BF16, tag="expS",
                                     bufs=4, name=f"es{h}_{t}_{cs}")
                        nc.scalar.activation(eS, S_p[:, 0:ce - cs], Act.Exp)
                        if cs == q0:
                            nc.vector.tensor_mul(eS[:, 0:128],
                                                 eS[:, 0:128], mk)
                        cur.append((h, t, eS, cs, ce))
                for (h, pt, e, cs, ce) in prev:
                    av(h, pt, e, cs, ce)
                prev = cur
            for (h, pt, e, cs, ce) in prev:
                av(h, pt, e, cs, ce)

            # normalize: denom rows -> expand-matmul broadcast -> recip -> mul
            D01 = sb.tile([65, 2048], BF16, tag="d01", bufs=1, name=f"d{kt}")
            nc.vector.tensor_copy(D01[64:65, 0:QL], O[h0][64:65, :])
            nc.vector.tensor_copy(D01[64:65, QL:2 * QL], O[h1][64:65, :])
            R0sb = sb.tile([64, QL], F32, tag="rsb", bufs=2, name=f"r0s{kt}")
            R1sb = sb.tile([64, QL], F32, tag="rsb", bufs=2, name=f"r1s{kt}")
            for hh, Rsb in ((0, R0sb), (1, R1sb)):
                for n in range(2):
                    Rp = pst_sc(f"r{kt}_{hh}_{n}")
                    nc.tensor.matmul(
                        Rp, E_sb[64:65, :],
                        D01[64:65, 1024 * hh + 512 * n:1024 * hh + 512 * n + 512],
                        start=True, stop=True)
                    nc.vector.reciprocal_approx_fast(
                        out=Rsb[:, 512 * n:512 * n + 512], in_=Rp[0:64, :])
            nc.vector.tensor_mul(attn_bf[0:64, kt, :], O[h0][0:64, :], R0sb)
            stg = sb.tile([64, QL], BF16, tag="stg", bufs=1, name=f"stg{kt}")
            nc.vector.tensor_mul(stg, O[h1][0:64, :], R1sb)
            nc.sync.dma_start(out=attn_bf[64:128, kt, :], in_=stg)

        # ---------- Wo + residual (bf16) ----------
        r_bf = sb.tile([128, KT, QL], BF16, tag="r")
        for mo in range(KT):
            wot = sb.tile([128, KT, 128], BF16, tag="wk6", bufs=2,
                          name=f"wo{mo}")
            nc.sync.dma_start(out=wot, in_=Wo[:, :, 128 * mo:128 * mo + 128])
            for n in range(QL // 512):
                cs = slice(512 * n, 512 * n + 512)
                pst = pst_tile(f"ops{mo}_{n}", (128, 512))
                for kt in range(KT):
                    nc.tensor.matmul(pst, wot[:, kt, :], attn_bf[:, kt, cs],
                                     start=(kt == 0), stop=(kt == KT - 1))
                if biases_zero:
                    nc.vector.tensor_add(r_bf[:, mo, cs], pst,
                                         xbT_sb[:, mo, cs])
                else:
                    nc.vector.scalar_tensor_tensor(r_bf[:, mo, cs], pst,
                                                   par[:, 36 + mo:37 + mo],
                                                   xbT_sb[:, mo, cs],
                                                   Alu.add, Alu.add)

        # prefetch W2 (gpsimd queue) into space freed by Q / attn / xbT
        W2a = sb.tile([128, 8, HID], BF16, tag="Q", name="W2a")
        W2b = sb.tile([128, 8, HID], BF16, tag="attn", name="W2b")
        W2c = sb.tile([128, 8, HID], BF16, tag="xbT", name="W2c")
        W2t = (W2a, W2b, W2c)
        for k2 in range(MH):
            nc.gpsimd.dma_start(out=W2t[k2 // 8][:, k2 % 8, :], in_=W2[k2])

        ln2_bf = sb.tile([128, KT, QL], BF16, tag="V", name="ln2")
        emit_ln(QL, r_bf, ln2_bf, "l2", f32_sq=True)

        # ---------- MLP: W1 mo-outer (resident), then W2 per cs-half ----------
        g0 = sb.tile([128, 12, QL], BF16, tag="ln", name="g0")
        g1 = sb.tile([128, 12, QL], BF16, tag="K", name="g1")
        for mo in range(MH):
            gt = g0 if mo < 12 else g1
            if mo < 12:
                w1t = W1_sb[:, mo, :, :]
            else:
                w1t = sb.tile([128, KT, 128], BF16, tag="wk6", bufs=2,
                              name=f"w1s{mo}")
                nc.gpsimd.dma_start(out=w1t, in_=W1[mo])
            for n in range(QL // 512):
                cs = slice(512 * n, 512 * n + 512)
                pst = pst_tile(f"h1ps{mo}_{n}", (128, 512))
                for kt in range(KT):
                    nc.tensor.matmul(pst, w1t[:, kt, :],
                                     ln2_bf[:, kt, cs],
                                     start=(kt == 0), stop=(kt == KT - 1))
                if biases_zero:
                    nc.scalar.activation(gt[:, mo % 12, cs], pst, Act.Gelu)
                else:
                    nc.scalar.activation(gt[:, mo % 12, cs], pst, Act.Gelu,
                                         bias=par[:, 48 + mo:49 + mo],
                                         scale=1.0)
        for n in range(QL // 512):
            cs = slice(512 * n, 512 * n + 512)
            pA = pst_O(f"ypsA{n}", (128, 1024))
            pB = pst_O(f"ypsB{n}", (128, 1024))
            pC = pst_sc(f"ypsC{n}")
            pD = pst_sc(f"ypsD{n}")
            slc = [pA[:, 0:512], pA[:, 512:1024], pB[:, 0:512],
                   pB[:, 512:1024], pC, pD]
            for k2 in range(MH):
                gt = g0 if k2 < 12 else g1
                for mo in range(KT):
                    nc.tensor.matmul(
                        slc[mo],
                        W2t[k2 // 8][:, k2 % 8, 128 * mo:128 * mo + 128],
                        gt[:, k2 % 12, cs],
                        start=(k2 == 0), stop=(k2 == MH - 1))
            for mo in range(KT):
                pslice = slc[mo]
                yst = sb.tile([128, 512], F32, tag="yst", bufs=2,
                              name=f"yst{n}_{mo}")
                if biases_zero:
                    nc.vector.tensor_add(yst, pslice, r_bf[:, mo, cs])
                else:
                    nc.vector.scalar_tensor_tensor(yst, pslice,
                                                   par[:, 42 + mo:43 + mo],
                                                   r_bf[:, mo, cs],
                                                   Alu.add, Alu.add)
                nc.sync.dma_start(out=y[128 * mo:128 * mo + 128, cs], in_=yst)

    nc.compile()
    return nc


def _get_program(biases_zero):
    key = ("nc", biases_zero)
    if key not in _CACHE:
        _CACHE[key] = _build_program(biases_zero)
    return _CACHE[key]


def _prep_in_maps(inputs):
    import ml_dtypes
    bf = ml_dtypes.bfloat16
    f32 = np.float32

    x = np.ascontiguousarray(np.asarray(inputs["x"], dtype=f32))
    ln1w = np.asarray(inputs["ln1_w"], f32)
    ln1b = np.asarray(inputs["ln1_b"], f32)
    ln2w = np.asarray(inputs["ln2_w"], f32)
    ln2b = np.asarray(inputs["ln2_b"], f32)
    Wq = np.asarray(inputs["Wq"], f32)
    Wk = np.asarray(inputs["Wk"], f32)
    Wv = np.asarray(inputs["Wv"], f32)
    Wo = np.asarray(inputs["Wo"], f32)
    W1 = np.asarray(inputs["W1"], f32)
    W2 = np.asarray(inputs["W2"], f32)
    # fold LN gains into consumer weights; LN bias contribution into proj
    # biases; fold the 1/sqrt(D) score scale into Wq
    Wq_f = ln1w[:, None] * Wq * np.float32(1.0 / np.sqrt(D))
    Wk_f = ln1w[:, None] * Wk
    Wv_f = ln1w[:, None] * Wv
    W1_f = ln2w[:, None] * W1
    bq_e = (Wq.T @ ln1b + np.asarray(inputs["bq"], f32)) / np.float32(np.sqrt(D))
    bk_e = Wk.T @ ln1b + np.asarray(inputs["bk"], f32)
    bv_e = Wv.T @ ln1b + np.asarray(inputs["bv"], f32)
    b1_e = W1.T @ ln2b + np.asarray(inputs["b1"], f32)
    bo_e = np.asarray(inputs["bo"], f32)
    b2_e = np.asarray(inputs["b2"], f32)
    biases_zero = bool(
        all(np.all(v == 0) for v in (bq_e, bk_e, bv_e, b1_e, bo_e, b2_e)))

    def pack_kp(W):  # [HID, M] -> [128, KT, M] with row k*128+p -> [p, k]
        M = W.shape[1]
        return np.ascontiguousarray(
            W.reshape(KT, 128, M).transpose(1, 0, 2).astype(bf))

    W1p = pack_kp(W1_f)                       # [128, KT, 3072]
    W1p = np.ascontiguousarray(
        W1p.reshape(128, KT, MH, 128).transpose(2, 0, 1, 3))  # [mo][p][kt][128]
    W2p = np.ascontiguousarray(
        W2.reshape(MH, 128, HID).astype(bf))  # [k2][p][m]

    shared = {
        "Wq": pack_kp(Wq_f),
        "Wk": pack_kp(Wk_f),
        "Wv": pack_kp(Wv_f),
        "Wo": pack_kp(Wo),
        "W1": W1p,
        "W2": W2p,
        "bqs": bq_e, "bk": bk_e, "bv": bv_e, "bo": bo_e,
        "b1": b1_e, "b2": b2_e,
    }

    in_maps = []
    qcols_all = []
    for c in range(N_CORES):
        b, p = c // 2, c % 2
        qcols = np.concatenate(
            [np.arange(128 * (2 * j + p), 128 * (2 * j + p) + 128)
             for j in range(8)])
        ocols = np.concatenate(
            [np.arange(128 * (2 * j + 1 - p), 128 * (2 * j + 1 - p) + 128)
             for j in range(8)])
        qcols_all.append(qcols)
        xp = np.concatenate([x[b][qcols], x[b][ocols]], axis=0)  # [S, HID]
        xbT = np.ascontiguousarray(
            xp.T.reshape(KT, 128, S).transpose(1, 0, 2).astype(bf))
        # masks: [tri (own-parity diagonal) | flat (other-parity first block)]
        m = np.zeros((2, 128, 128), np.float32)
        kk = np.arange(128)[:, None]
        qq = np.arange(128)[None, :]
        m[0] = (kk <= qq).astype(np.float32)
        m[1] = 0.0 if p == 0 else 1.0
        mperm = np.ascontiguousarray(m.transpose(1, 0, 2))  # [128, 2, 128]
        im = dict(shared)
        im["xbT"] = xbT
        im["masks"] = mperm.astype(bf)
        in_maps.append(im)
    return in_maps, qcols_all, biases_zero


def kernel(**inputs):
    import sys, types
    if "antenv.axon_hooks" not in sys.modules:
        try:
            sys.path.insert(0, "/root/.axon_site")
            from trn_agent_boot.trn_boot import _ntff_profile_via_ctypes
            hook = _ntff_profile_via_ctypes("/opt/axon/libaxon_pjrt.so")
            mod = types.ModuleType("antenv.axon_hooks")
            mod.get_axon_ntff_profile_hook = lambda: hook
            mod.set_axon_ntff_profile_hook = lambda h: None
            import antenv  # noqa: F401
            sys.modules["antenv.axon_hooks"] = mod
        except Exception:
            pass

    from concourse.bass_utils import run_bass_kernel_spmd

    in_maps, qcols_all, biases_zero = _prep_in_maps(inputs)
    nc = _get_program(biases_zero)
    res = run_bass_kernel_spmd(nc, in_maps, core_ids=list(range(N_CORES)))
    out = np.zeros((B, S, HID), np.float32)
    for c in range(N_CORES):
        out[c // 2, qcols_all[c], :] = res.results[c]["y"].T
    return out


# revision 34
# speedup vs baseline: 1.0957x; 1.0957x over previous
"""Trainium2 Bass kernel for a pre-LN transformer block (B=4, S=2048, H=12, D=64).

Sharding: 8 cores; core c -> batch b = c//2, parity p = c%2.
Each core handles the 1024 query rows of its batch whose 128-token block index
has parity p (stride-2 interleave balances causal load; SPMD shared program).

v2 layout trick: the host PERMUTES each core's token axis so that the core's
own query tokens are columns 0:1024 and the other parity's tokens are columns
1024:2048.  Q projection / residual / output then just use the first half of
the feature-major activations; causal structure is carried entirely by the
per-core host-computed multiplicative masks (triangular for key tiles 0..7,
all-0 / all-1 for key tiles 8..15 depending on parity).

Other v2 changes vs baseline:
- All weights host-packed into per-partition-contiguous layouts, streamed
  mo-outer so each weight is DMA'd once at high efficiency (W1/W2 prefetched
  into SBUF on the gpsimd queue during attention / Wo).
- LayerNorm stays on-chip: ones-matmul sums -> 1-lane math on [1,512] ->
  outer-product matmul broadcast into PSUM -> DVE apply.  No DRAM bounces.
- Softmax: exp on Scalar with head-pair merged activations when the span fits
  512 cols; denominators from the ones-row in V, broadcast by a row-64
  expand-matmul, normalized by DVE reciprocal+multiply.  No DRAM bounces.
- Residual r kept in bf16; biases folded on host (all zero for this model).
"""

import numpy as np

N_CORES = 8
B, S, H, D = 4, 2048, 12, 64
HID = 768
QL = 1024
KT = HID // 128     # 6 feature blocks
TT = S // 128       # 16 key tiles
MH = 4 * HID // 128  # 24 hidden blocks
EPS = 1e-5

_CACHE = {}


def _build_program(biases_zero, debug=False):
    from contextlib import ExitStack
    import concourse.bass as bass
    import concourse.tile as tile
    from concourse import bacc, mybir

    F32 = mybir.dt.float32
    BF16 = mybir.dt.bfloat16
    Alu = mybir.AluOpType
    Act = mybir.ActivationFunctionType

    nc = bacc.Bacc("TRN2", target_bir_lowering=False, debug=False,
                   enable_asserts=False, num_devices=N_CORES)

    def din(name, shape, dt):
        return nc.dram_tensor(name, shape, dt, kind="ExternalInput").ap()

    # activations / consts (per-core)
    xbT = din("xbT", [128, KT, S], BF16)       # packed, token-permuted
    masks = din("masks", [128, 2, 128], BF16)  # [tri | flat(parity)]
    # weights (shared across cores), host-packed per-partition-contiguous
    Wq = din("Wq", [128, KT, HID], BF16)       # [p][kt][m]; ln1_w and 1/8 folded
    Wk = din("Wk", [128, KT, HID], BF16)       # ln1_w folded
    Wv = din("Wv", [128, KT, HID], BF16)       # ln1_w folded
    Wo = din("Wo", [128, KT, HID], BF16)
    W1 = din("W1", [MH, 128, KT, 128], BF16)   # [mo][p][kt][cols]; ln2_w folded
    W2 = din("W2", [MH, 128, HID], BF16)       # [k2][p][m]
    bqs = din("bqs", [HID], F32)
    bk = din("bk", [HID], F32)
    bv = din("bv", [HID], F32)
    bo = din("bo", [HID], F32)
    b1 = din("b1", [4 * HID], F32)
    b2 = din("b2", [HID], F32)

    y = nc.dram_tensor("y", [HID, QL], F32, kind="ExternalOutput").ap()

    def bcast(src_elem_ap, parts, n):
        return bass.AP(tensor=src_elem_ap.tensor, offset=src_elem_ap.offset,
                       ap=[[0, parts], [1, n]])

    with tile.TileContext(nc) as tc, ExitStack() as ctx:
        sb = ctx.enter_context(tc.tile_pool(name="sb", bufs=1))
        ps = ctx.enter_context(tc.tile_pool(name="ps", bufs=1, space="PSUM"))

        def pst_tile(name, shape=(128, 1024)):
            return ps.tile(list(shape), F32, tag="s2", bufs=4, name=name,
                           padded_shape=[128, 1024])

        # ---------- constants ----------
        ones_bf = sb.tile([128, 1], BF16, tag="ones")
        nc.vector.memset(ones_bf, 1.0)
        ones_f = sb.tile([128, 1], F32, tag="onesf")
        nc.vector.memset(ones_f, 1.0)
        ones_row = sb.tile([1, 128], BF16, tag="onesr")
        nc.vector.memset(ones_row, 1.0)
        par = sb.tile([128, 80], F32, tag="par")

        def load_cols(dst0, src, n):
            nc.gpsimd.dma_start(
                out=par[:, dst0:dst0 + n],
                in_=bass.AP(tensor=src.tensor, offset=src.offset,
                            ap=[[1, 128], [128, n]]))

        load_cols(24, bqs, KT)
        load_cols(30, bk, KT)
        load_cols(36, bo, KT)
        load_cols(42, b2, KT)
        load_cols(48, b1, MH)
        nc.vector.memset(par[:, 72:73], EPS)
        eps_t = par[:, 72:73]
        if not biases_zero:
            bv_b = sb.tile([128, HID], F32, tag="bv_b")
            nc.gpsimd.dma_start(out=bv_b, in_=bcast(bv[0], 128, HID))
        E_sb = sb.tile([65, 128], BF16, tag="emat")
        nc.vector.memset(E_sb[64:65, :], 1.0)
        masks_sb = sb.tile([128, 2, 128], BF16, tag="masks")
        nc.gpsimd.dma_start(out=masks_sb, in_=masks)

        # PE warmup: lift the HAM clock gate while input DMAs land
        warm = sb.tile([128, 512], BF16, tag="sq", bufs=2, name="warm")
        nc.vector.memset(warm, 0.0)
        wps = pst_tile("warmps", (1, 512))
        for i in range(8):
            nc.tensor.matmul(wps, ones_bf, warm, start=True, stop=True)

        # ---------- input load ----------
        xbT_sb = sb.tile([128, KT, S], BF16, tag="xbT")
        for c in range(S // 512):
            nc.sync.dma_start(out=xbT_sb[:, :, 512 * c:512 * c + 512],
                              in_=xbT[:, :, 512 * c:512 * c + 512])

        # ---------- LN helper: broadcast stats first, then 128-lane math ------
        def emit_ln(N, x_bf, out_bf, pfx, f32_sq=False):
            for c in range(N // 512):
                off = 512 * c
                cs = slice(off, off + 512)
                s_ps = pst_tile(f"{pfx}s{c}", (1, 512))
                q_ps = pst_tile(f"{pfx}q{c}", (1, 512))
                for kt in range(KT):
                    nc.tensor.matmul(s_ps, ones_bf, x_bf[:, kt, cs],
                                     start=(kt == 0), stop=(kt == KT - 1))
                for kt in range(KT):
                    sqd = F32 if f32_sq else BF16
                    sqc = sb.tile([128, 512], sqd, tag="sq", bufs=2,
                                  name=f"{pfx}sq{c}_{kt}")
                    nc.vector.tensor_mul(sqc, x_bf[:, kt, cs], x_bf[:, kt, cs])
                    nc.tensor.matmul(q_ps, ones_f if f32_sq else ones_bf, sqc,
                                     start=(kt == 0), stop=(kt == KT - 1))
                s_b = sb.tile([1, 512], BF16, tag="st1b", bufs=2, name=f"{pfx}sb{c}")
                q_b = sb.tile([1, 512], BF16, tag="st1b", bufs=2, name=f"{pfx}qb{c}")
                nc.scalar.copy(s_b, s_ps)
                nc.scalar.copy(q_b, q_ps)
                S_ps = pst_tile(f"{pfx}S{c}", (128, 512))
                Q_ps = pst_tile(f"{pfx}Q{c}", (128, 512))
                nc.tensor.matmul(S_ps, ones_row, s_b, start=True, stop=True)
                nc.tensor.matmul(Q_ps, ones_row, q_b, start=True, stop=True)
                # full-width math: var = q/H - mu^2; rstd = 1/sqrt(var+eps)
                mu = sb.tile([128, 512], F32, tag="lnm", bufs=6, name=f"{pfx}mu{c}")
                var = sb.tile([128, 512], F32, tag="lnm", bufs=6, name=f"{pfx}v{c}")
                nc.scalar.mul(mu, S_ps, 1.0 / HID)
                nc.scalar.mul(var, Q_ps, 1.0 / HID)
                nmu2 = sb.tile([128, 512], F32, tag="lnm", bufs=6,
                               name=f"{pfx}n2{c}")
                nc.vector.scalar_tensor_tensor(nmu2, mu, -1.0, mu, Alu.mult,
                                               Alu.mult)  # -mu^2
                nc.vector.tensor_add(var, var, nmu2)      # q/H - mu^2
                sd = sb.tile([128, 512], F32, tag="lnm", bufs=6, name=f"{pfx}sd{c}")
                nc.scalar.activation(sd, var, Act.Sqrt, bias=eps_t, scale=1.0)
                a_t = sb.tile([128, 512], F32, tag="lnm", bufs=6, name=f"{pfx}a{c}")
                nc.vector.reciprocal_approx_fast(out=a_t, in_=sd)
                c_t = sb.tile([128, 512], F32, tag="lnm", bufs=6, name=f"{pfx}c{c}")
                nc.vector.scalar_tensor_tensor(c_t, mu, -1.0, a_t, Alu.mult,
                                               Alu.mult)  # -mu * rstd
                for kt in range(KT):
                    t0 = sb.tile([128, 512], BF16, tag="t0", bufs=2,
                                 name=f"{pfx}t0{c}_{kt}")
                    nc.vector.tensor_mul(t0, x_bf[:, kt, cs], a_t)
                    nc.vector.tensor_add(out_bf[:, kt, cs], t0, c_t)

        ln_bf = sb.tile([128, KT, S], BF16, tag="ln")
        emit_ln(S, xbT_sb, ln_bf, "l1")

        # evacuation helper: psum -> sbuf (+ optional bias col)
        def evac(dst, src, bias_col=None):
            if biases_zero or bias_col is None:
                nc.scalar.copy(dst, src)
            else:
                nc.vector.tensor_scalar(dst, src, 1.0, bias_col,
                                        Alu.mult, Alu.add)

        # ---------- QKV projections (mo-outer, weights loaded once) ----------
        K_sb = sb.tile([128, KT, S], BF16, tag="K")
        Q_sb = sb.tile([128, KT, QL], BF16, tag="Q")
        for mo in range(KT):
            wkt = sb.tile([128, KT, 128], BF16, tag="wk6", bufs=2,
                          name=f"wk{mo}")
            nc.sync.dma_start(out=wkt, in_=Wk[:, :, 128 * mo:128 * mo + 128])
            for n in range(S // 512):
                cs = slice(512 * n, 512 * n + 512)
                pst = pst_tile(f"kps{mo}_{n}", (128, 512))
                for kt in range(KT):
                    nc.tensor.matmul(pst, wkt[:, kt, :], ln_bf[:, kt, cs],
                                     start=(kt == 0), stop=(kt == KT - 1))
                evac(K_sb[:, mo, cs], pst, par[:, 30 + mo:31 + mo])
        for mo in range(KT):
            wqt = sb.tile([128, KT, 128], BF16, tag="wk6", bufs=2,
                          name=f"wq{mo}")
            nc.sync.dma_start(out=wqt, in_=Wq[:, :, 128 * mo:128 * mo + 128])
            for n in range(QL // 512):
                cs = slice(512 * n, 512 * n + 512)
                pst = pst_tile(f"qps{mo}_{n}", (128, 512))
                for kt in range(KT):
                    nc.tensor.matmul(pst, wqt[:, kt, :], ln_bf[:, kt, cs],
                                     start=(kt == 0), stop=(kt == KT - 1))
                evac(Q_sb[:, mo, cs], pst, par[:, 24 + mo:25 + mo])

        V_sb = sb.tile([128, H, TT, 65], BF16, tag="V")
        for h in range(H):
            nc.vector.memset(V_sb[:, h, :, 64:65], 1.0)
        for fc in range(2):
            wvt = sb.tile([128, KT, 384], BF16, tag="wv", bufs=1, name=f"wv{fc}")
            nc.sync.dma_start(out=wvt,
                              in_=Wv[:, :, 384 * fc:384 * fc + 384])
            for tt in range(TT):
                pst = pst_tile(f"vps{tt}_{fc}", (128, 384))
                for kt in range(KT):
                    nc.tensor.matmul(pst, ln_bf[:, kt, 128 * tt:128 * tt + 128],
                                     wvt[:, kt, :],
                                     start=(kt == 0), stop=(kt == KT - 1))
                vdst = V_sb[:, 6 * fc:6 * fc + 6, tt, 0:64]
                if biases_zero:
                    nc.scalar.copy(vdst, pst.rearrange("p (h d) -> p h d", d=64))
                else:
                    nc.vector.tensor_tensor(
                        vdst, pst.rearrange("p (h d) -> p h d", d=64),
                        bv_b[:, 384 * fc:384 * fc + 384].rearrange(
                            "p (h d) -> p h d", d=64),
                        Alu.add)

        # prefetch half of W1 during attention (gpsimd queue, off critical path);
        # the other half streams per-mo during the MLP
        W1_sb = sb.tile([128, 12, KT, 128], BF16, tag="W1")
        for mo in range(12):
            nc.gpsimd.dma_start(out=W1_sb[:, mo, :, :], in_=W1[mo])

        # ---------- attention ----------
        attn_bf = sb.tile([128, KT, QL], BF16, tag="attn")
        for kt in range(KT):
            h0, h1 = 2 * kt, 2 * kt + 1
            O = {h0: pst_tile(f"o{h0}", (65, QL)),
                 h1: pst_tile(f"o{h1}", (65, QL))}

            def av(h, tile_t, e, q0):
                for (cs, ce) in ([(q0, 512), (512, QL)] if q0 < 512
                                 else [(q0, QL)]):
                    nc.tensor.matmul(O[h][:, cs:ce],
                                     V_sb[:, h, tile_t, :],
                                     e[:, cs - q0:ce - q0],
                                     start=(tile_t == 0),
                                     stop=(tile_t == TT - 1))

            prev = []
            for t in range(TT):
                q0 = 128 * (t % 8)
                span = QL - q0
                cur = []
                if span <= 512:
                    # merged head-pair scores: h0 at cols 0:span, h1 at 512:
                    S_m = pst_tile(f"sm{kt}_{t}", (128, 1024))
                    for h, pr, co in ((h0, slice(0, 64), 0),
                                      (h1, slice(64, 128), 512)):
                        nc.tensor.matmul(S_m[:, co:co + span],
                                         K_sb[pr, kt, 128 * t:128 * t + 128],
                                         Q_sb[pr, kt, q0:QL],
                                         start=True, stop=True)
                    eS = sb.tile([128, 512 + span], BF16, tag="expS", bufs=4,
                                 name=f"es{kt}_{t}")
                    nc.scalar.activation(eS, S_m[:, 0:512 + span], Act.Exp)
                    for h, co in ((h0, 0), (h1, 512)):
                        nc.vector.tensor_mul(eS[:, co:co + 128],
                                             eS[:, co:co + 128],
                                             masks_sb[:, 0 if t < 8 else 1, :])
                        cur.append((h, eS[:, co:co + span], q0, t))
                else:
                    for h, pr in ((h0, slice(0, 64)), (h1, slice(64, 128))):
                        S_ps = pst_tile(f"sc{h}_{t}", (128, QL))
                        for (cs, ce) in ([(q0, 512), (512, QL)] if q0 < 512
                                         else [(q0, QL)]):
                            nc.tensor.matmul(
                                S_ps[:, cs:ce],
                                K_sb[pr, kt, 128 * t:128 * t + 128],
                                Q_sb[pr, kt, cs:ce], start=True, stop=True)
                        eS = sb.tile([128, span], BF16, tag="expS", bufs=4,
                                     name=f"es{h}_{t}")
                        nc.scalar.activation(eS, S_ps[:, q0:QL], Act.Exp)
                        nc.vector.tensor_mul(eS[:, 0:128], eS[:, 0:128],
                                             masks_sb[:, 0 if t < 8 else 1, :])
                        cur.append((h, eS, q0, t))
                for (h, e, pq0, pt) in prev:
                    av(h, pt, e, pq0)
                prev = cur
            for (h, e, pq0, pt) in prev:
                av(h, pt, e, pq0)

            # normalize: denom rows -> expand-matmul broadcast -> recip -> mul
            D01 = sb.tile([65, 2048], BF16, tag="d01", bufs=1, name=f"d{kt}")
            nc.vector.tensor_copy(D01[64:65, 0:QL], O[h0][64:65, :])
            nc.vector.tensor_copy(D01[64:65, QL:2 * QL], O[h1][64:65, :])
            R0 = pst_tile(f"r0{kt}", (128, QL))
            R1 = pst_tile(f"r1{kt}", (128, QL))
            for n in range(2):
                cs = slice(512 * n, 512 * n + 512)
                nc.tensor.matmul(R0[:, cs], E_sb[64:65, :],
                                 D01[64:65, cs], start=True, stop=True)
                nc.tensor.matmul(
                    R1[:, cs], E_sb[64:65, :],
                    D01[64:65, 1024 + 512 * n:1024 + 512 * n + 512],
                    start=True, stop=True)
            R0sb = sb.tile([64, QL], F32, tag="rsb", bufs=2, name=f"r0s{kt}")
            R1sb = sb.tile([64, QL], F32, tag="rsb", bufs=2, name=f"r1s{kt}")
            nc.vector.reciprocal_approx_fast(out=R0sb, in_=R0[0:64, :])
            nc.vector.reciprocal_approx_fast(out=R1sb, in_=R1[0:64, :])
            nc.vector.tensor_mul(attn_bf[0:64, kt, :], O[h0][0:64, :], R0sb)
            stg = sb.tile([64, QL], BF16, tag="stg", bufs=1, name=f"stg{kt}")
            nc.vector.tensor_mul(stg, O[h1][0:64, :], R1sb)
            nc.sync.dma_start(out=attn_bf[64:128, kt, :], in_=stg)

        # ---------- Wo + residual (bf16) ----------
        r_bf = sb.tile([128, KT, QL], BF16, tag="r")
        for mo in range(KT):
            wot = sb.tile([128, KT, 128], BF16, tag="wk6", bufs=2,
                          name=f"wo{mo}")
            nc.sync.dma_start(out=wot, in_=Wo[:, :, 128 * mo:128 * mo + 128])
            for n in range(QL // 512):
                cs = slice(512 * n, 512 * n + 512)
                pst = pst_tile(f"ops{mo}_{n}", (128, 512))
                for kt in range(KT):
                    nc.tensor.matmul(pst, wot[:, kt, :], attn_bf[:, kt, cs],
                                     start=(kt == 0), stop=(kt == KT - 1))
                if biases_zero:
                    nc.vector.tensor_add(r_bf[:, mo, cs], pst,
                                         xbT_sb[:, mo, cs])
                else:
                    nc.vector.scalar_tensor_tensor(r_bf[:, mo, cs], pst,
                                                   par[:, 36 + mo:37 + mo],
                                                   xbT_sb[:, mo, cs],
                                                   Alu.add, Alu.add)

        # prefetch W2 (gpsimd queue) into space freed by Q / attn / xbT
        W2a = sb.tile([128, 8, HID], BF16, tag="Q", name="W2a")
        W2b = sb.tile([128, 8, HID], BF16, tag="attn", name="W2b")
        W2c = sb.tile([128, 8, HID], BF16, tag="xbT", name="W2c")
        W2t = (W2a, W2b, W2c)
        for k2 in range(MH):
            nc.gpsimd.dma_start(out=W2t[k2 // 8][:, k2 % 8, :], in_=W2[k2])

        ln2_bf = sb.tile([128, KT, QL], BF16, tag="V", name="ln2")
        emit_ln(QL, r_bf, ln2_bf, "l2", f32_sq=True)

        # ---------- MLP: W1 mo-outer (resident), then W2 per cs-half ----------
        g0 = sb.tile([128, 12, QL], BF16, tag="ln", name="g0")
        g1 = sb.tile([128, 12, QL], BF16, tag="K", name="g1")
        for mo in range(MH):
            gt = g0 if mo < 12 else g1
            if mo < 12:
                w1t = W1_sb[:, mo, :, :]
            else:
                w1t = sb.tile([128, KT, 128], BF16, tag="wk6", bufs=2,
                              name=f"w1s{mo}")
                nc.gpsimd.dma_start(out=w1t, in_=W1[mo])
            for n in range(QL // 512):
                cs = slice(512 * n, 512 * n + 512)
                pst = pst_tile(f"h1ps{mo}_{n}", (128, 512))
                for kt in range(KT):
                    nc.tensor.matmul(pst, w1t[:, kt, :],
                                     ln2_bf[:, kt, cs],
                                     start=(kt == 0), stop=(kt == KT - 1))
                if biases_zero:
                    nc.scalar.activation(gt[:, mo % 12, cs], pst, Act.Gelu)
                else:
                    nc.scalar.activation(gt[:, mo % 12, cs], pst, Act.Gelu,
                                         bias=par[:, 48 + mo:49 + mo],
                                         scale=1.0)
        for n in range(QL // 512):
            cs = slice(512 * n, 512 * n + 512)
            psts = [pst_tile(f"yps{n}_{i}") for i in range(3)]
            for k2 in range(MH):
                gt = g0 if k2 < 12 else g1
                for mo in range(KT):
                    nc.tensor.matmul(
                        psts[mo // 2][:, 512 * (mo % 2):512 * (mo % 2) + 512],
                        W2t[k2 // 8][:, k2 % 8, 128 * mo:128 * mo + 128],
                        gt[:, k2 % 12, cs],
                        start=(k2 == 0), stop=(k2 == MH - 1))
            for mo in range(KT):
                pslice = psts[mo // 2][:, 512 * (mo % 2):512 * (mo % 2) + 512]
                yst = sb.tile([128, 512], F32, tag="yst", bufs=2,
                              name=f"yst{n}_{mo}")
                if biases_zero:
                    nc.vector.tensor_add(yst, pslice, r_bf[:, mo, cs])
                else:
                    nc.vector.scalar_tensor_tensor(yst, pslice,
                                                   par[:, 42 + mo:43 + mo],
                                                   r_bf[:, mo, cs],
                                                   Alu.add, Alu.add)
                nc.sync.dma_start(out=y[128 * mo:128 * mo + 128, cs], in_=yst)

    nc.compile()
    return nc


def _get_program(biases_zero):
    key = ("nc", biases_zero)
    if key not in _CACHE:
        _CACHE[key] = _build_program(biases_zero)
    return _CACHE[key]


def _prep_in_maps(inputs):
    import ml_dtypes
    bf = ml_dtypes.bfloat16
    f32 = np.float32

    x = np.ascontiguousarray(np.asarray(inputs["x"], dtype=f32))
    ln1w = np.asarray(inputs["ln1_w"], f32)
    ln1b = np.asarray(inputs["ln1_b"], f32)
    ln2w = np.asarray(inputs["ln2_w"], f32)
    ln2b = np.asarray(inputs["ln2_b"], f32)
    Wq = np.asarray(inputs["Wq"], f32)
    Wk = np.asarray(inputs["Wk"], f32)
    Wv = np.asarray(inputs["Wv"], f32)
    Wo = np.asarray(inputs["Wo"], f32)
    W1 = np.asarray(inputs["W1"], f32)
    W2 = np.asarray(inputs["W2"], f32)
    # fold LN gains into consumer weights; LN bias contribution into proj
    # biases; fold the 1/sqrt(D) score scale into Wq
    Wq_f = ln1w[:, None] * Wq * np.float32(1.0 / np.sqrt(D))
    Wk_f = ln1w[:, None] * Wk
    Wv_f = ln1w[:, None] * Wv
    W1_f = ln2w[:, None] * W1
    bq_e = (Wq.T @ ln1b + np.asarray(inputs["bq"], f32)) / np.float32(np.sqrt(D))
    bk_e = Wk.T @ ln1b + np.asarray(inputs["bk"], f32)
    bv_e = Wv.T @ ln1b + np.asarray(inputs["bv"], f32)
    b1_e = W1.T @ ln2b + np.asarray(inputs["b1"], f32)
    bo_e = np.asarray(inputs["bo"], f32)
    b2_e = np.asarray(inputs["b2"], f32)
    biases_zero = bool(
        all(np.all(v == 0) for v in (bq_e, bk_e, bv_e, b1_e, bo_e, b2_e)))

    def pack_kp(W):  # [HID, M] -> [128, KT, M] with row k*128+p -> [p, k]
        M = W.shape[1]
        return np.ascontiguousarray(
            W.reshape(KT, 128, M).transpose(1, 0, 2).astype(bf))

    W1p = pack_kp(W1_f)                       # [128, KT, 3072]
    W1p = np.ascontiguousarray(
        W1p.reshape(128, KT, MH, 128).transpose(2, 0, 1, 3))  # [mo][p][kt][128]
    W2p = np.ascontiguousarray(
        W2.reshape(MH, 128, HID).astype(bf))  # [k2][p][m]

    shared = {
        "Wq": pack_kp(Wq_f),
        "Wk": pack_kp(Wk_f),
        "Wv": pack_kp(Wv_f),
        "Wo": pack_kp(Wo),
        "W1": W1p,
        "W2": W2p,
        "bqs": bq_e, "bk": bk_e, "bv": bv_e, "bo": bo_e,
        "b1": b1_e, "b2": b2_e,
    }

    in_maps = []
    qcols_all = []
    for c in range(N_CORES):
        b, p = c // 2, c % 2
        qcols = np.concatenate(
            [np.arange(128 * (2 * j + p), 128 * (2 * j + p) + 128)
             for j in range(8)])
        ocols = np.concatenate(
            [np.arange(128 * (2 * j + 1 - p), 128 * (2 * j + 1 - p) + 128)
             for j in range(8)])
        qcols_all.append(qcols)
        xp = np.concatenate([x[b][qcols], x[b][ocols]], axis=0)  # [S, HID]
        xbT = np.ascontiguousarray(
            xp.T.reshape(KT, 128, S).transpose(1, 0, 2).astype(bf))
        # masks: [tri (own-parity diagonal) | flat (other-parity first block)]
        m = np.zeros((2, 128, 128), np.float32)
        kk = np.arange(128)[:, None]
        qq = np.arange(128)[None, :]
        m[0] = (kk <= qq).astype(np.float32)
        m[1] = 0.0 if p == 0 else 1.0
        mperm = np.ascontiguousarray(m.transpose(1, 0, 2))  # [128, 2, 128]
        im = dict(shared)
        im["xbT"] = xbT
        im["masks"] = mperm.astype(bf)
        in_maps.append(im)
    return in_maps, qcols_all, biases_zero


def kernel(**inputs):
    import sys, types
    if "antenv.axon_hooks" not in sys.modules:
        try:
            sys.path.insert(0, "/root/.axon_site")
            from trn_agent_boot.trn_boot import _ntff_profile_via_ctypes
            hook = _ntff_profile_via_ctypes("/opt/axon/libaxon_pjrt.so")
            mod = types.ModuleType("antenv.axon_hooks")
            mod.get_axon_ntff_profile_hook = lambda: hook
            mod.set_axon_ntff_profile_hook = lambda h: None
            import antenv  # noqa: F401
            sys.modules["antenv.axon_hooks"] = mod
        except Exception:
            pass

    from concourse.bass_utils import run_bass_kernel_spmd

    in_maps, qcols_all, biases_zero = _prep_in_maps(inputs)
    nc = _get_program(biases_zero)
    res = run_bass_kernel_spmd(nc, in_maps, core_ids=list(range(N_CORES)))
    out = np.zeros((B, S, HID), np.float32)
    for c in range(N_CORES):
        out[c // 2, qcols_all[c], :] = res.results[c]["y"].T
    return out
